# revision 25
# baseline (speedup 1.0000x reference)
"""Trainium2 Bass kernel for im2col Conv2d dot-product:
out[b, n] = <enc_x[b, n, :], w_flat> + bias.

Data-parallel over batch: 8 batches per NeuronCore x 8 cores.

TensorEngine split-K formulation (fp16). PSUM cell (m, n) of an
accumulation group sums contributions from column n of EVERY matmul in
the group, so a group of 49 matmuls x 128 rows gives 6272 row-slots per
column position: exactly 128 windows x 49 taps. Window m of column-block
n has its taps spread across the group's matmuls at flat slot
s = m*49 + k -> (matmul t = s//128, row r = s%128):

  stationary_t[r, s//49] = w[s%49]   (s = t*128 + r; one nonzero per row)
  rhs_t[r, n]            = x[window(g, s//49, n), s%49]
  psum[m, n]  +=  over t  ->  full dot of window  g*65536 + m*512 + n

The host pre-arranges x (cast to fp16) so each core reads one flat
[128, 153664] tensor: group-major, then matmul-major, then column --
every DMA is a full-128-partition contiguous load. 6 full groups of
49 matmuls at FD=512 (65536 windows each) + 1 partial group at FD=64.
The Scalar engine drains PSUM -> SBUF with a fused bias add; one
contiguous [128, 512] store per group.

Per core: 39.3 MB fp16 in at the ~360 GB/s HBM-per-NC roofline
(~105 us), PE ~70-100 us, DVE/GpSimd idle. fp16 rounding of x and w
gives rel err ~3e-4 vs the fp32 reference (tolerance 2e-2); products
accumulate in fp32 PSUM.
"""

from contextlib import ExitStack

import numpy as np

import concourse.bass as bass
import concourse.tile as tile
from concourse import mybir

B = 64
WINDOWS = 50176
K = 49
NCORES = 8
BPC = B // NCORES            # batches per core
NWIN = BPC * WINDOWS         # 401408 windows per core

MM_PER_G = 49                # matmuls per PSUM accumulation group
NFULL = 6                    # full groups: 128 x 512 windows each
FULL_N = 512                 # columns (free dim) per full-group matmul
PART_N = 64                  # columns of the final partial group
GROUPW = 128 * FULL_N        # 65536 windows per full group
PARTW = 128 * PART_N         # 8192 windows in the partial group
assert NFULL * GROUPW + PARTW == NWIN

FULL_COLS = MM_PER_G * FULL_N   # 25088 elems per partition per full group
PART_COLS = MM_PER_G * PART_N   # 3136
XCOLS = NFULL * FULL_COLS + PART_COLS  # 153664 fp16 per partition per core

# Flat chunk plan (in matmuls), decoupled from PSUM-group boundaries:
# small ramp chunks so the PE starts early, 32-MM (4.2 MB) steady chunks
# for maximal per-DMA efficiency, then a geometric taper. The taper ratio
# respects PE 216 ns/MM vs DMA 333 ns/MM (m_{k+1} >= 0.65 m_k): the PE
# finishes each chunk as the next lands, so after the last byte only ~2
# matmuls + drain remain. Sums to 343 matmuls.
CHUNK_PLAN = (4, 8, 16, 25, 48, 48, 48, 48, 32, 24, 16, 10, 7, 4, 3, 2)
# stationaries split: first 16 load first (0.5 MB) so matmul 0 starts ~11us
WS_SPLIT = 16
XBUFS = 3

FP32 = mybir.dt.float32
FP16 = mybir.dt.float16

_NC = None


def _build_nc():
    nc = bass.Bass(trn_type="TRN2", debug=False, num_devices=NCORES)

    xh = nc.dram_tensor("xh", [128, XCOLS], FP16, kind="ExternalInput").ap()
    ws = nc.dram_tensor("ws", [128, MM_PER_G * 128], FP16,
                        kind="ExternalInput").ap()
    b = nc.dram_tensor("b", [1], FP32, kind="ExternalInput").ap()
    out = nc.dram_tensor("out", [NWIN], FP16, kind="ExternalOutput").ap()

    with tile.TileContext(nc) as tc, ExitStack() as ctx:
        consts = ctx.enter_context(tc.tile_pool(name="consts", bufs=1))
        xpool = ctx.enter_context(tc.tile_pool(name="x", bufs=XBUFS))
        pspool = ctx.enter_context(tc.tile_pool(name="ps", bufs=2,
                                                space="PSUM"))
        opool = ctx.enter_context(tc.tile_pool(name="o", bufs=2))

        # Stationaries load first on the sync ring, split so matmul 0 only
        # waits for the small first piece; bias rides the otherwise-idle
        # scalar ring.
        wst0 = consts.tile([128, WS_SPLIT * 128], FP16)
        wst1 = consts.tile([128, (MM_PER_G - WS_SPLIT) * 128], FP16)
        nc.sync.dma_start(out=wst0[:],
                          in_=bass.AP(tensor=ws.tensor, offset=ws.offset,
                                      ap=[[MM_PER_G * 128, 128],
                                          [1, WS_SPLIT * 128]]))
        bb = consts.tile([128, 1], FP32)
        nc.scalar.dma_start(
            out=bb[:],
            in_=bass.AP(tensor=b.tensor, offset=b.offset,
                        ap=[[0, 128]] + list(b.ap)),
        )

        def stationary(t):
            if t < WS_SPLIT:
                return wst0[:, t * 128:(t + 1) * 128]
            return wst1[:, (t - WS_SPLIT) * 128:(t - WS_SPLIT + 1) * 128]

        # Flat matmul sequence in issue order: group 0, the partial group
        # (so its 49 short matmuls are not the kernel tail), groups 1..5.
        # The host lays xh columns out in this same order.
        seq = []
        for gid, fd in ([(0, FULL_N), (NFULL, PART_N)]
                        + [(g, FULL_N) for g in range(1, NFULL)]):
            for t in range(MM_PER_G):
                seq.append((gid, t, fd))

        def drain(ps, gid, ncols, name):
            # fp16 output halves the store traffic; the host upcasts to
            # fp32 (adds ~5e-4 rel rounding vs the 2e-2 gate)
            ot = opool.tile([128, ncols], FP16, tag="ot", name=name)
            nc.scalar.activation(
                out=ot[:], in_=ps[:],
                func=mybir.ActivationFunctionType.Identity,
                bias=bb[:, 0:1], scale=1.0,
            )
            dst = bass.AP(tensor=out.tensor,
                          offset=out.offset + gid * GROUPW,
                          ap=[[ncols, 128], [1, ncols]])
            nc.scalar.dma_start(out=dst, in_=ot[:])

        mi = 0          # next matmul in seq
        col = 0         # xh column cursor
        ps = None
        for ci, cmms in enumerate(CHUNK_PLAN):
            ncols = sum(fd for _, _, fd in seq[mi:mi + cmms])
            xt = xpool.tile([128, ncols], FP16, tag="xt", name=f"xt{ci}")
            src = bass.AP(tensor=xh.tensor, offset=xh.offset + col,
                          ap=[[XCOLS, 128], [1, ncols]])
            nc.sync.dma_start(out=xt[:], in_=src)
            if ci == 0:
                # second stationary piece queues right behind chunk 0
                nc.sync.dma_start(
                    out=wst1[:],
                    in_=bass.AP(tensor=ws.tensor,
                                offset=ws.offset + WS_SPLIT * 128,
                                ap=[[MM_PER_G * 128, 128],
                                    [1, (MM_PER_G - WS_SPLIT) * 128]]))
            off = 0
            for gid, t, fd in seq[mi:mi + cmms]:
                if t == 0:
                    ps = pspool.tile([128, fd], FP32, tag="ps",
                                     name=f"ps{gid}")
                nc.tensor.matmul(
                    ps[:],
                    lhsT=stationary(t),
                    rhs=xt[:, off:off + fd],
                    start=(t == 0),
                    stop=(t == MM_PER_G - 1),
                )
                if t == MM_PER_G - 1:
                    drain(ps, gid, fd, f"ot{gid}")
                off += fd
            col += ncols
            mi += cmms
        assert mi == len(seq) and col == XCOLS

    return nc


def _split_ctrl_waits(nc, max_waits=1):
    """Work around a walrus codegen limit on this build: instructions accept
    only one sync-wait command. Hoist extra waits onto dedicated no-op
    instructions inserted just before, preserving per-engine order."""
    from concourse import mybir

    for f in nc.m.functions:
        for blk in f.blocks:
            insts = blk.instructions
            i = 0
            while i < len(insts):
                ins = insts[i]
                if (
                    ins.sync_info is not None
                    and len(ins.sync_info.on_wait) > max_waits
                ):
                    waits = list(ins.sync_info.on_wait)
                    keep, extra = waits[:max_waits], waits[max_waits:]
                    ins.sync_info.on_wait = keep
                    for j, wchunk in enumerate(extra):
                        nop = mybir.InstNoOp(
                            name=f"{ins.name}-wsplit{j}",
                            sync_info=mybir.SyncInfo(on_wait=[wchunk], on_update=[]),
                            bass_nofuse=True,
                            engine=ins.engine,
                        )
                        nc.register_instruction(nop, overwrite=True)
                        insts.insert(i, nop)
                        i += 1
                i += 1


def _patch_ldw_opt():
    """Experimental: flip walrus --enable-ldw-opt to true (gated by env)."""
    import os
    import concourse.bass_utils as bu

    if not os.environ.get("KERNEL_LDW_OPT"):
        return
    if getattr(bu, "_ldw_patched", False):
        return
    orig = bu.bir_verify_and_optimise

    def patched(*a, **kw):
        real_run = bu.run_command

        def run2(cmd, **k):
            cmd = ["--enable-ldw-opt=true" if c == "--enable-ldw-opt=false"
                   else c for c in cmd]
            return real_run(cmd, **k)

        bu.run_command = run2
        try:
            return orig(*a, **kw)
        finally:
            bu.run_command = real_run

    bu.bir_verify_and_optimise = patched
    bu._ldw_patched = True


def _get_nc():
    global _NC
    if _NC is None:
        _patch_ldw_opt()
        _NC = _build_nc()
        _split_ctrl_waits(_NC)
    return _NC


def _host_prep(enc_x, weight, bias):
    """Cast to fp16 and pre-arrange per-core tensors for the split-K PE
    formulation (see module docstring for the layout)."""
    xf = np.asarray(enc_x, dtype=np.float32).reshape(NCORES, NWIN, K)
    x16 = xf.astype(np.float16)

    def block(xs, ncol):
        # [m, n, k] -> flat slot s = m*49+k rows: [s, n] -> [t, r, n]
        xg = xs.reshape(128, ncol, K)
        y = xg.transpose(0, 2, 1).reshape(MM_PER_G * 128, ncol)
        z = y.reshape(MM_PER_G, 128, ncol).transpose(1, 0, 2)
        return z.reshape(128, MM_PER_G * ncol)

    def core_layout(xc):
        # column order matches the kernel's matmul issue order:
        # group 0, partial group, groups 1..5
        parts = [block(xc[0:GROUPW], FULL_N),
                 block(xc[NFULL * GROUPW:], PART_N)]
        for g in range(1, NFULL):
            parts.append(block(xc[g * GROUPW:(g + 1) * GROUPW], FULL_N))
        return np.concatenate(parts, axis=1)

    xh = np.stack([core_layout(x16[i]) for i in range(NCORES)], axis=0)
    xh = np.ascontiguousarray(xh)

    w49 = np.asarray(weight, dtype=np.float32).reshape(K).astype(np.float16)
    ws = np.zeros((128, MM_PER_G * 128), dtype=np.float16)
    s = np.arange(MM_PER_G * 128)
    t, r = s // 128, s % 128
    m, k = s // K, s % K
    ws[r, t * 128 + m] = w49[k]

    bf = np.asarray(bias, dtype=np.float32).reshape(1)
    return xh, ws, bf


def run(enc_x, weight, bias, trace=False, **spmd_kwargs):
    """Run on 8 NeuronCores; returns (out [B, WINDOWS] fp32, BassKernelResults)."""
    from concourse.bass_utils import run_bass_kernel_spmd

    nc = _get_nc()
    xh, ws, bf = _host_prep(enc_x, weight, bias)
    in_maps = [{"xh": xh[i], "ws": ws, "b": bf} for i in range(NCORES)]
    res = run_bass_kernel_spmd(
        nc, in_maps, list(range(NCORES)), trace=trace, **spmd_kwargs
    )
    out = np.stack([res.results[i]["out"] for i in range(NCORES)], axis=0)
    return out.reshape(B, WINDOWS).astype(np.float32), res


def kernel(enc_x, weight, bias, windows_nb=None):
    out, _ = run(enc_x, weight, bias)
    return out


# revision 27
# speedup vs baseline: 1.0345x; 1.0345x over previous
"""Trainium2 Bass kernel for im2col Conv2d dot-product:
out[b, n] = <enc_x[b, n, :], w_flat> + bias.

Data-parallel over batch: 8 batches per NeuronCore x 8 cores.

TensorEngine split-K formulation (fp16). PSUM cell (m, n) of an
accumulation group sums contributions from column n of EVERY matmul in
the group, so a group of 49 matmuls x 128 rows gives 6272 row-slots per
column position: exactly 128 windows x 49 taps. Window m of column-block
n has its taps spread across the group's matmuls at flat slot
s = m*49 + k -> (matmul t = s//128, row r = s%128):

  stationary_t[r, s//49] = w[s%49]   (s = t*128 + r; one nonzero per row)
  rhs_t[r, n]            = x[window(g, s//49, n), s%49]
  psum[m, n]  +=  over t  ->  full dot of window  g*65536 + m*512 + n

The host pre-arranges x (cast to fp16) so each core reads one flat
[128, 153664] tensor: group-major, then matmul-major, then column --
every DMA is a full-128-partition contiguous load. 6 full groups of
49 matmuls at FD=512 (65536 windows each) + 1 partial group at FD=64.
The Scalar engine drains PSUM -> SBUF with a fused bias add; one
contiguous [128, 512] store per group.

Per core: 39.3 MB fp16 in at the ~360 GB/s HBM-per-NC roofline
(~105 us), PE ~70-100 us, DVE/GpSimd idle. fp16 rounding of x and w
gives rel err ~3e-4 vs the fp32 reference (tolerance 2e-2); products
accumulate in fp32 PSUM.
"""

from contextlib import ExitStack

import numpy as np

import concourse.bass as bass
import concourse.tile as tile
from concourse import mybir

B = 64
WINDOWS = 50176
K = 49
NCORES = 8
BPC = B // NCORES            # batches per core
NWIN = BPC * WINDOWS         # 401408 windows per core

MM_PER_G = 49                # matmuls per PSUM accumulation group
NFULL = 6                    # full groups: 128 x 512 windows each
FULL_N = 512                 # columns (free dim) per full-group matmul
PART_N = 64                  # columns of the final partial group
GROUPW = 128 * FULL_N        # 65536 windows per full group
PARTW = 128 * PART_N         # 8192 windows in the partial group
assert NFULL * GROUPW + PARTW == NWIN

FULL_COLS = MM_PER_G * FULL_N   # 25088 elems per partition per full group
PART_COLS = MM_PER_G * PART_N   # 3136
XCOLS = NFULL * FULL_COLS + PART_COLS  # 153664 fp16 per partition per core

# Flat chunk plan (in matmuls), decoupled from PSUM-group boundaries:
# small ramp chunks so the PE starts early, 32-MM (4.2 MB) steady chunks
# for maximal per-DMA efficiency, then a geometric taper. The taper ratio
# respects PE 216 ns/MM vs DMA 333 ns/MM (m_{k+1} >= 0.65 m_k): the PE
# finishes each chunk as the next lands, so after the last byte only ~2
# matmuls + drain remain. Sums to 343 matmuls.
CHUNK_PLAN = (4, 8, 16, 25) + (32,) * 7 + (24, 16, 10, 7, 4, 3, 2)
# stationaries split: first 16 load first (0.5 MB) so matmul 0 starts ~11us
WS_SPLIT = 16
XBUFS = 4

FP32 = mybir.dt.float32
FP16 = mybir.dt.float16

_NC = None


def _build_nc():
    nc = bass.Bass(trn_type="TRN2", debug=False, num_devices=NCORES)

    xh = nc.dram_tensor("xh", [128, XCOLS], FP16, kind="ExternalInput").ap()
    ws = nc.dram_tensor("ws", [128, MM_PER_G * 128], FP16,
                        kind="ExternalInput").ap()
    b = nc.dram_tensor("b", [1], FP32, kind="ExternalInput").ap()
    out = nc.dram_tensor("out", [NWIN], FP16, kind="ExternalOutput").ap()

    with tile.TileContext(nc) as tc, ExitStack() as ctx:
        consts = ctx.enter_context(tc.tile_pool(name="consts", bufs=1))
        xpool = ctx.enter_context(tc.tile_pool(name="x", bufs=XBUFS))
        pspool = ctx.enter_context(tc.tile_pool(name="ps", bufs=2,
                                                space="PSUM"))
        opool = ctx.enter_context(tc.tile_pool(name="o", bufs=2))

        # Stationaries load first on the sync ring, split so matmul 0 only
        # waits for the small first piece; bias rides the otherwise-idle
        # scalar ring.
        wst0 = consts.tile([128, WS_SPLIT * 128], FP16)
        wst1 = consts.tile([128, (MM_PER_G - WS_SPLIT) * 128], FP16)
        nc.sync.dma_start(out=wst0[:],
                          in_=bass.AP(tensor=ws.tensor, offset=ws.offset,
                                      ap=[[MM_PER_G * 128, 128],
                                          [1, WS_SPLIT * 128]]))
        bb = consts.tile([128, 1], FP32)
        nc.scalar.dma_start(
            out=bb[:],
            in_=bass.AP(tensor=b.tensor, offset=b.offset,
                        ap=[[0, 128]] + list(b.ap)),
        )

        def stationary(t):
            if t < WS_SPLIT:
                return wst0[:, t * 128:(t + 1) * 128]
            return wst1[:, (t - WS_SPLIT) * 128:(t - WS_SPLIT + 1) * 128]

        # Flat matmul sequence in issue order: group 0, the partial group
        # (so its 49 short matmuls are not the kernel tail), groups 1..5.
        # The host lays xh columns out in this same order.
        seq = []
        for gid, fd in ([(0, FULL_N), (NFULL, PART_N)]
                        + [(g, FULL_N) for g in range(1, NFULL)]):
            for t in range(MM_PER_G):
                seq.append((gid, t, fd))

        def drain(ps, gid, ncols, name):
            # fp16 output halves the store traffic; the host upcasts to
            # fp32 (adds ~5e-4 rel rounding vs the 2e-2 gate)
            ot = opool.tile([128, ncols], FP16, tag="ot", name=name)
            nc.scalar.activation(
                out=ot[:], in_=ps[:],
                func=mybir.ActivationFunctionType.Identity,
                bias=bb[:, 0:1], scale=1.0,
            )
            dst = bass.AP(tensor=out.tensor,
                          offset=out.offset + gid * GROUPW,
                          ap=[[ncols, 128], [1, ncols]])
            nc.scalar.dma_start(out=dst, in_=ot[:])

        mi = 0          # next matmul in seq
        col = 0         # xh column cursor
        ps = None
        for ci, cmms in enumerate(CHUNK_PLAN):
            ncols = sum(fd for _, _, fd in seq[mi:mi + cmms])
            xt = xpool.tile([128, ncols], FP16, tag="xt", name=f"xt{ci}")
            src = bass.AP(tensor=xh.tensor, offset=xh.offset + col,
                          ap=[[XCOLS, 128], [1, ncols]])
            nc.sync.dma_start(out=xt[:], in_=src)
            if ci == 0:
                # second stationary piece queues right behind chunk 0
                nc.sync.dma_start(
                    out=wst1[:],
                    in_=bass.AP(tensor=ws.tensor,
                                offset=ws.offset + WS_SPLIT * 128,
                                ap=[[MM_PER_G * 128, 128],
                                    [1, (MM_PER_G - WS_SPLIT) * 128]]))
            off = 0
            for gid, t, fd in seq[mi:mi + cmms]:
                if t == 0:
                    ps = pspool.tile([128, fd], FP32, tag="ps",
                                     name=f"ps{gid}")
                nc.tensor.matmul(
                    ps[:],
                    lhsT=stationary(t),
                    rhs=xt[:, off:off + fd],
                    start=(t == 0),
                    stop=(t == MM_PER_G - 1),
                )
                if t == MM_PER_G - 1:
                    drain(ps, gid, fd, f"ot{gid}")
                off += fd
            col += ncols
            mi += cmms
        assert mi == len(seq) and col == XCOLS

    return nc


def _split_ctrl_waits(nc, max_waits=1):
    """Work around a walrus codegen limit on this build: instructions accept
    only one sync-wait command. Hoist extra waits onto dedicated no-op
    instructions inserted just before, preserving per-engine order."""
    from concourse import mybir

    for f in nc.m.functions:
        for blk in f.blocks:
            insts = blk.instructions
            i = 0
            while i < len(insts):
                ins = insts[i]
                if (
                    ins.sync_info is not None
                    and len(ins.sync_info.on_wait) > max_waits
                ):
                    waits = list(ins.sync_info.on_wait)
                    keep, extra = waits[:max_waits], waits[max_waits:]
                    ins.sync_info.on_wait = keep
                    for j, wchunk in enumerate(extra):
                        nop = mybir.InstNoOp(
                            name=f"{ins.name}-wsplit{j}",
                            sync_info=mybir.SyncInfo(on_wait=[wchunk], on_update=[]),
                            bass_nofuse=True,
                            engine=ins.engine,
                        )
                        nc.register_instruction(nop, overwrite=True)
                        insts.insert(i, nop)
                        i += 1
                i += 1


def _patch_ldw_opt():
    """Experimental: flip walrus --enable-ldw-opt to true (gated by env)."""
    import os
    import concourse.bass_utils as bu

    if not os.environ.get("KERNEL_LDW_OPT"):
        return
    if getattr(bu, "_ldw_patched", False):
        return
    orig = bu.bir_verify_and_optimise

    def patched(*a, **kw):
        real_run = bu.run_command

        def run2(cmd, **k):
            cmd = ["--enable-ldw-opt=true" if c == "--enable-ldw-opt=false"
                   else c for c in cmd]
            return real_run(cmd, **k)

        bu.run_command = run2
        try:
            return orig(*a, **kw)
        finally:
            bu.run_command = real_run

    bu.bir_verify_and_optimise = patched
    bu._ldw_patched = True


def _get_nc():
    global _NC
    if _NC is None:
        _patch_ldw_opt()
        _NC = _build_nc()
        _split_ctrl_waits(_NC)
    return _NC


def _host_prep(enc_x, weight, bias):
    """Cast to fp16 and pre-arrange per-core tensors for the split-K PE
    formulation (see module docstring for the layout)."""
    xf = np.asarray(enc_x, dtype=np.float32).reshape(NCORES, NWIN, K)
    x16 = xf.astype(np.float16)

    def block(xs, ncol):
        # [m, n, k] -> flat slot s = m*49+k rows: [s, n] -> [t, r, n]
        xg = xs.reshape(128, ncol, K)
        y = xg.transpose(0, 2, 1).reshape(MM_PER_G * 128, ncol)
        z = y.reshape(MM_PER_G, 128, ncol).transpose(1, 0, 2)
        return z.reshape(128, MM_PER_G * ncol)

    def core_layout(xc):
        # column order matches the kernel's matmul issue order:
        # group 0, partial group, groups 1..5
        parts = [block(xc[0:GROUPW], FULL_N),
                 block(xc[NFULL * GROUPW:], PART_N)]
        for g in range(1, NFULL):
            parts.append(block(xc[g * GROUPW:(g + 1) * GROUPW], FULL_N))
        return np.concatenate(parts, axis=1)

    xh = np.stack([core_layout(x16[i]) for i in range(NCORES)], axis=0)
    xh = np.ascontiguousarray(xh)

    w49 = np.asarray(weight, dtype=np.float32).reshape(K).astype(np.float16)
    ws = np.zeros((128, MM_PER_G * 128), dtype=np.float16)
    s = np.arange(MM_PER_G * 128)
    t, r = s // 128, s % 128
    m, k = s // K, s % K
    ws[r, t * 128 + m] = w49[k]

    bf = np.asarray(bias, dtype=np.float32).reshape(1)
    return xh, ws, bf


def run(enc_x, weight, bias, trace=False, **spmd_kwargs):
    """Run on 8 NeuronCores; returns (out [B, WINDOWS] fp32, BassKernelResults)."""
    from concourse.bass_utils import run_bass_kernel_spmd

    nc = _get_nc()
    xh, ws, bf = _host_prep(enc_x, weight, bias)
    in_maps = [{"xh": xh[i], "ws": ws, "b": bf} for i in range(NCORES)]
    res = run_bass_kernel_spmd(
        nc, in_maps, list(range(NCORES)), trace=trace, **spmd_kwargs
    )
    out = np.stack([res.results[i]["out"] for i in range(NCORES)], axis=0)
    return out.reshape(B, WINDOWS).astype(np.float32), res


def kernel(enc_x, weight, bias, windows_nb=None):
    out, _ = run(enc_x, weight, bias)
    return out
